# revision 2
# baseline (speedup 1.0000x reference)
"""Bass/Trainium2 kernel for nn_KnowledgeD2V (doc2vec NCE loss).

Computation (see reference):
  doc_ids = input_labels[:, -1]; ctx = input_labels[:, :-1]
  x = doc_embed[doc_ids] + word_embed[ctx].sum(1)              # [B, D]
  y = out_embed[[out_labels, noise]]                           # [B, 1+S, D]
  s = einsum('bd,bkd->bk', x, y)                               # [B, 1+S]
  loss = mean_b( softplus(-s[:,0]) + sum_k softplus(s[:,k>0]) )

Strategy: data-parallel over batch across 8 NeuronCores. The host gathers the
32 embedding rows each batch element touches (20 ctx + 1 doc + 1 target + 10
noise) into one contiguous bf16 stream `packed[B, 4096]` — the target row is
sign-flipped so every loss term is softplus(+s) and the device needs a single
Exp/Ln pair per group. Each core streams its 2048 rows tile-by-tile (128
batch rows per SBUF partition-tile, 8 KB/partition sequential HWDGE DMA at
full line rate), then does ALL the arithmetic on-device:
  - x = sum of the 21 ctx+doc rows, via a log-tree of TensorTensor adds
    (contiguous bf16, hits the DVE 2x packed mode; TensorReduce has no fast
    mode so trees beat reductions),
  - prod = y * x (broadcast TT, 2x),
  - dot-fold prod 128->64->32->16->8 (TT 2x) + final TensorReduce,
  - softplus via one Exp + one Ln(1+e) with accum_out -> per-group partials
    (a manual LoadActFuncSet of the shared Exp+Ln table stops the act-table
    pass from thrashing tables every tile),
  - partition-sum via a ones-vector PE matmul.
The host sums the 8 scalars / B. Indirect (gather) DMA is deliberately not
used: multi-offset indirect DMA mis-executes under this runtime.
"""

import numpy as np
import ml_dtypes

import concourse.bacc as bacc
import concourse.mybir as mybir
import concourse.tile as tile
from concourse import bass_utils

B = 16384
S = 10
K = 1 + S         # 11 score columns
D = 128
P = 128
NCORES = 8
BC = B // NCORES  # 2048 rows per core
T = BC // P       # 16 tiles per core
NWIN = 21         # ctx(20) + doc rows summed into x
CTX = NWIN * D    # 2688
FREE = CTX + K * D  # 4096 bf16 elems per batch row
U = 2             # tiles per DVE instruction group
GROUPS = [1, 1] + [U] * ((T - 2) // U)

F32 = mybir.dt.float32
BF16 = mybir.dt.bfloat16


def _build():
    nc = bacc.Bacc(
        "TRN2", target_bir_lowering=False, debug=False, num_devices=NCORES
    )
    packed_d = nc.dram_tensor("packed", [BC, FREE], BF16, kind="ExternalInput").ap()
    res_d = nc.dram_tensor("partial", [1, 1], F32, kind="ExternalOutput").ap()

    with tile.TileContext(nc) as tc:
        with (
            tc.tile_pool(name="setup", bufs=1) as sp,
            tc.tile_pool(name="work", bufs=3) as wp,
            tc.tile_pool(name="psum", bufs=1, space="PSUM") as pp,
        ):
            ones = sp.tile([P, 1], F32)
            nc.vector.memset(ones[:], 1.0)
            acc = sp.tile([P, len(GROUPS)], F32)

            # Preload the single ACT table that holds BOTH Exp and Ln so the
            # act-table-load pass does not reload a table per activation.
            from concourse.hw_specs import get_activation_tables
            _tabs = list(get_activation_tables(nc.m.arch).values())
            _EXP = mybir.ActivationFunctionType.Exp
            _LN = mybir.ActivationFunctionType.Ln
            _set_id = next(i for i, fs in enumerate(_tabs) if _EXP in fs and _LN in fs)
            nc.scalar.add_instruction(mybir.InstLoadActFuncSet(
                name=nc.get_next_instruction_name(),
                act_func_set_id=_set_id, ins=[], outs=[]))

            def tt(out_t, a, b, op=mybir.AluOpType.add):
                nc.vector.tensor_tensor(out=out_t, in0=a, in1=b, op=op)

            t_next = 0
            for gi, GU in enumerate(GROUPS):
                tiles = [t_next + u for u in range(GU)]
                t_next += GU
                g = wp.tile([P, GU, FREE], BF16, tag=f"g{GU}")
                for u, ti in enumerate(tiles):
                    nc.sync.dma_start(
                        out=g[:, u, :], in_=packed_d[ti * P : (ti + 1) * P, :]
                    )

                with nc.allow_low_precision(reason="bf16 tree-sum of 21 embeds"):
                    r1 = wp.tile([P, GU, 10 * D], BF16, tag=f"r1_{GU}")
                    tt(r1[:], g[:, :, 0 : 10 * D], g[:, :, 10 * D : 20 * D])
                    r2 = wp.tile([P, GU, 5 * D], BF16, tag=f"r2_{GU}")
                    tt(r2[:], r1[:, :, 0 : 5 * D], r1[:, :, 5 * D : 10 * D])
                    r3 = wp.tile([P, GU, 2 * D], BF16, tag=f"r3_{GU}")
                    tt(r3[:], r2[:, :, 0 : 2 * D], r2[:, :, 2 * D : 4 * D])
                    r4 = wp.tile([P, GU, D], BF16, tag=f"r4_{GU}")
                    tt(r4[:], r3[:, :, 0:D], r3[:, :, D : 2 * D])
                    r5 = wp.tile([P, GU, D], BF16, tag=f"r5_{GU}")
                    tt(r5[:], r4[:], r2[:, :, 4 * D : 5 * D])
                    x = wp.tile([P, GU, D], BF16, tag=f"x_{GU}")
                    tt(x[:], r5[:], g[:, :, 20 * D : 21 * D])

                prod = wp.tile([P, GU, K, D], BF16, tag=f"prod_{GU}")
                nc.vector.tensor_tensor(
                    out=prod[:],
                    in0=g[:, :, CTX:FREE].rearrange("p u (k d) -> p u k d", d=D),
                    in1=x[:].rearrange("p u (o d) -> p u o d", o=1)
                         .to_broadcast([P, GU, K, D]),
                    op=mybir.AluOpType.mult,
                )
                with nc.allow_low_precision(reason="bf16 dot folds; loss avgs 180K terms"):
                    f1 = wp.tile([P, GU, K, 64], BF16, tag=f"f1_{GU}")
                    tt(f1[:], prod[:, :, :, 0:64], prod[:, :, :, 64:128])
                    f2 = wp.tile([P, GU, K, 32], BF16, tag=f"f2_{GU}")
                    tt(f2[:], f1[:, :, :, 0:32], f1[:, :, :, 32:64])
                    f3 = wp.tile([P, GU, K, 16], BF16, tag=f"f3_{GU}")
                    tt(f3[:], f2[:, :, :, 0:16], f2[:, :, :, 16:32])
                    f4 = wp.tile([P, GU, K, 8], BF16, tag=f"f4_{GU}")
                    tt(f4[:], f3[:, :, :, 0:8], f3[:, :, :, 8:16])
                v = wp.tile([P, GU * K], F32, tag=f"v_{GU}")
                nc.vector.reduce_sum(
                    out=v[:].rearrange("p (u k) -> p u k", u=GU),
                    in_=f4[:], axis=mybir.AxisListType.X)

                e = wp.tile([P, GU * K], F32, tag=f"e_{GU}")
                nc.scalar.activation(out=e[:], in_=v[:],
                                     func=mybir.ActivationFunctionType.Exp)
                spt = wp.tile([P, GU * K], F32, tag=f"sp_{GU}")
                nc.scalar.activation(out=spt[:], in_=e[:],
                                     func=mybir.ActivationFunctionType.Ln, bias=1.0,
                                     accum_out=acc[:, gi : gi + 1])

            total = sp.tile([P, 1], F32)
            nc.vector.reduce_sum(out=total[:], in_=acc[:], axis=mybir.AxisListType.X)
            ps = pp.tile([1, 1], F32, space="PSUM")
            nc.tensor.matmul(out=ps[:], lhsT=total[:], rhs=ones[:], start=True, stop=True)
            res_sb = sp.tile([1, 1], F32)
            nc.vector.tensor_copy(out=res_sb[:], in_=ps[:])
            nc.sync.dma_start(out=res_d, in_=res_sb[:])

    nc.compile()
    return nc


_NC = None
_LAST_RESULTS = None  # BassKernelResults of the most recent run (for test harness)


def _get_nc():
    global _NC
    if _NC is None:
        _NC = _build()
    return _NC


def _prep(inputs):
    il = np.asarray(inputs["input_labels"]).astype(np.int64)
    ol = np.asarray(inputs["out_labels"]).astype(np.int64)
    nz = np.asarray(inputs["noise"]).astype(np.int64)
    we = np.asarray(inputs["word_embed"], dtype=np.float32)
    oe = np.asarray(inputs["out_embed"], dtype=np.float32)
    de = np.asarray(inputs["doc_embed"], dtype=np.float32)
    assert int(inputs["num_sampled"]) == S

    bf = ml_dtypes.bfloat16
    packed = np.empty((B, FREE), dtype=bf)
    # 20 ctx rows + 1 doc row -> x part
    packed[:, 0 : 20 * D] = we[il[:, 0:20]].reshape(B, -1).astype(bf)
    packed[:, 20 * D : CTX] = de[il[:, 20]].astype(bf)
    # y part: negated target row first, then the 10 noise rows
    packed[:, CTX : CTX + D] = (-oe[ol]).astype(bf)
    packed[:, CTX + D :] = oe[nz].reshape(B, -1).astype(bf)
    return packed


def kernel(**inputs) -> np.ndarray:
    packed = _prep(inputs)
    nc = _get_nc()
    in_maps = [
        {"packed": packed[c * BC : (c + 1) * BC]} for c in range(NCORES)
    ]
    res = bass_utils.run_bass_kernel_spmd(nc, in_maps, core_ids=list(range(NCORES)))
    global _LAST_RESULTS
    _LAST_RESULTS = res
    total = sum(float(r["partial"][0, 0]) for r in res.results)
    return np.float32(total / B)


# revision 4
# speedup vs baseline: 1.0154x; 1.0154x over previous
"""Bass/Trainium2 kernel for nn_KnowledgeD2V (doc2vec NCE loss).

Computation (see reference):
  doc_ids = input_labels[:, -1]; ctx = input_labels[:, :-1]
  x = doc_embed[doc_ids] + word_embed[ctx].sum(1)              # [B, D]
  y = out_embed[[out_labels, noise]]                           # [B, 1+S, D]
  s = einsum('bd,bkd->bk', x, y)                               # [B, 1+S]
  loss = mean_b( softplus(-s[:,0]) + sum_k softplus(s[:,k>0]) )

Strategy: data-parallel over batch across 8 NeuronCores. The host gathers the
32 embedding rows each batch element touches (20 ctx + 1 doc + 1 target + 10
noise) into one contiguous bf16 stream `packed[B, 4096]` — the target row is
sign-flipped so every loss term is softplus(+s) and the device needs a single
Exp/Ln pair per group. Each core streams its 2048 rows tile-by-tile (128
batch rows per SBUF partition-tile, 8 KB/partition sequential HWDGE DMA at
full line rate), then does ALL the arithmetic on-device:
  - x = sum of the 21 ctx+doc rows, via a log-tree of TensorTensor adds
    (contiguous bf16, hits the DVE 2x packed mode; TensorReduce has no fast
    mode so trees beat reductions),
  - prod = y * x (broadcast TT, 2x),
  - dot-fold prod 128->64->32->16->8 (TT 2x) + final TensorReduce,
  - softplus via one Exp + one Ln(1+e) with accum_out -> per-group partials
    (a manual LoadActFuncSet of the shared Exp+Ln table stops the act-table
    pass from thrashing tables every tile),
  - partition-sum via a ones-vector PE matmul.
The host sums the 8 scalars / B. Indirect (gather) DMA is deliberately not
used: multi-offset indirect DMA mis-executes under this runtime.
"""

import numpy as np
import ml_dtypes

import concourse.bacc as bacc
import concourse.mybir as mybir
import concourse.tile as tile
from concourse import bass_utils

B = 16384
S = 10
K = 1 + S         # 11 score columns
D = 128
P = 128
NCORES = 8
BC = B // NCORES  # 2048 rows per core
T = BC // P       # 16 tiles per core
NWIN = 21         # ctx(20) + doc rows summed into x
CTX = NWIN * D    # 2688
FREE = CTX + K * D  # 4096 bf16 elems per batch row
U = 2             # tiles per DVE instruction group
GROUPS = [1, 1] + [U] * ((T - 2) // U)

F32 = mybir.dt.float32
BF16 = mybir.dt.bfloat16


def _build():
    nc = bacc.Bacc(
        "TRN2", target_bir_lowering=False, debug=False, num_devices=NCORES
    )
    packed_d = nc.dram_tensor("packed", [BC, FREE], BF16, kind="ExternalInput").ap()
    res_d = nc.dram_tensor("partial", [1, 1], F32, kind="ExternalOutput").ap()

    with tile.TileContext(nc) as tc:
        with (
            tc.tile_pool(name="setup", bufs=1) as sp,
            tc.tile_pool(name="work", bufs=3) as wp,
            tc.tile_pool(name="psum", bufs=1, space="PSUM") as pp,
        ):
            ones = sp.tile([P, 1], F32)
            nc.vector.memset(ones[:], 1.0)
            acc = sp.tile([P, len(GROUPS)], F32)

            # Preload the single ACT table that holds BOTH Exp and Ln so the
            # act-table-load pass does not reload a table per activation.
            from concourse.hw_specs import get_activation_tables
            _tabs = list(get_activation_tables(nc.m.arch).values())
            _EXP = mybir.ActivationFunctionType.Exp
            _LN = mybir.ActivationFunctionType.Ln
            _set_id = next(i for i, fs in enumerate(_tabs) if _EXP in fs and _LN in fs)
            nc.scalar.add_instruction(mybir.InstLoadActFuncSet(
                name=nc.get_next_instruction_name(),
                act_func_set_id=_set_id, ins=[], outs=[]))

            def tt(out_t, a, b, op=mybir.AluOpType.add):
                nc.vector.tensor_tensor(out=out_t, in0=a, in1=b, op=op)

            t_next = 0
            for gi, GU in enumerate(GROUPS):
                tiles = [t_next + u for u in range(GU)]
                t_next += GU
                g = wp.tile([P, GU, FREE], BF16, tag=f"g{GU}", bufs=4)
                for u, ti in enumerate(tiles):
                    rows = packed_d[ti * P : (ti + 1) * P, :]
                    # split ctx/y so the x-tree can start after the ctx bytes
                    nc.sync.dma_start(out=g[:, u, 0:CTX], in_=rows[:, 0:CTX])
                    nc.sync.dma_start(out=g[:, u, CTX:FREE], in_=rows[:, CTX:FREE])

                with nc.allow_low_precision(reason="bf16 tree-sum of 21 embeds"):
                    r1 = wp.tile([P, GU, 10 * D], BF16, tag=f"r1_{GU}", bufs=2)
                    tt(r1[:], g[:, :, 0 : 10 * D], g[:, :, 10 * D : 20 * D])
                    r2 = wp.tile([P, GU, 5 * D], BF16, tag=f"r2_{GU}", bufs=2)
                    tt(r2[:], r1[:, :, 0 : 5 * D], r1[:, :, 5 * D : 10 * D])
                    r3 = wp.tile([P, GU, 2 * D], BF16, tag=f"r3_{GU}", bufs=2)
                    tt(r3[:], r2[:, :, 0 : 2 * D], r2[:, :, 2 * D : 4 * D])
                    r4 = wp.tile([P, GU, D], BF16, tag=f"r4_{GU}", bufs=2)
                    tt(r4[:], r3[:, :, 0:D], r3[:, :, D : 2 * D])
                    r5 = wp.tile([P, GU, D], BF16, tag=f"r5_{GU}", bufs=2)
                    tt(r5[:], r4[:], r2[:, :, 4 * D : 5 * D])
                    x = wp.tile([P, GU, D], BF16, tag=f"x_{GU}", bufs=2)
                    tt(x[:], r5[:], g[:, :, 20 * D : 21 * D])

                prod = wp.tile([P, GU, K, D], BF16, tag=f"prod_{GU}", bufs=2)
                nc.vector.tensor_tensor(
                    out=prod[:],
                    in0=g[:, :, CTX:FREE].rearrange("p u (k d) -> p u k d", d=D),
                    in1=x[:].rearrange("p u (o d) -> p u o d", o=1)
                         .to_broadcast([P, GU, K, D]),
                    op=mybir.AluOpType.mult,
                )
                with nc.allow_low_precision(reason="bf16 dot folds; loss avgs 180K terms"):
                    f1 = wp.tile([P, GU, K, 64], BF16, tag=f"f1_{GU}", bufs=2)
                    tt(f1[:], prod[:, :, :, 0:64], prod[:, :, :, 64:128])
                    f2 = wp.tile([P, GU, K, 32], BF16, tag=f"f2_{GU}", bufs=2)
                    tt(f2[:], f1[:, :, :, 0:32], f1[:, :, :, 32:64])
                    f3 = wp.tile([P, GU, K, 16], BF16, tag=f"f3_{GU}", bufs=2)
                    tt(f3[:], f2[:, :, :, 0:16], f2[:, :, :, 16:32])
                    f4 = wp.tile([P, GU, K, 8], BF16, tag=f"f4_{GU}", bufs=2)
                    tt(f4[:], f3[:, :, :, 0:8], f3[:, :, :, 8:16])
                v = wp.tile([P, GU * K], F32, tag=f"v_{GU}", bufs=2)
                nc.vector.reduce_sum(
                    out=v[:].rearrange("p (u k) -> p u k", u=GU),
                    in_=f4[:], axis=mybir.AxisListType.X)

                e = wp.tile([P, GU * K], F32, tag=f"e_{GU}", bufs=2)
                nc.scalar.activation(out=e[:], in_=v[:],
                                     func=mybir.ActivationFunctionType.Exp)
                spt = wp.tile([P, GU * K], F32, tag=f"sp_{GU}", bufs=2)
                nc.scalar.activation(out=spt[:], in_=e[:],
                                     func=mybir.ActivationFunctionType.Ln, bias=1.0,
                                     accum_out=acc[:, gi : gi + 1])

            total = sp.tile([P, 1], F32)
            nc.vector.reduce_sum(out=total[:], in_=acc[:], axis=mybir.AxisListType.X)
            ps = pp.tile([1, 1], F32, space="PSUM")
            nc.tensor.matmul(out=ps[:], lhsT=total[:], rhs=ones[:], start=True, stop=True)
            res_sb = sp.tile([1, 1], F32)
            nc.vector.tensor_copy(out=res_sb[:], in_=ps[:])
            nc.sync.dma_start(out=res_d, in_=res_sb[:])

    nc.compile()
    return nc


_NC = None
_LAST_RESULTS = None  # BassKernelResults of the most recent run (for test harness)


def _get_nc():
    global _NC
    if _NC is None:
        _NC = _build()
    return _NC


def _prep(inputs):
    il = np.asarray(inputs["input_labels"]).astype(np.int64)
    ol = np.asarray(inputs["out_labels"]).astype(np.int64)
    nz = np.asarray(inputs["noise"]).astype(np.int64)
    we = np.asarray(inputs["word_embed"], dtype=np.float32)
    oe = np.asarray(inputs["out_embed"], dtype=np.float32)
    de = np.asarray(inputs["doc_embed"], dtype=np.float32)
    assert int(inputs["num_sampled"]) == S

    bf = ml_dtypes.bfloat16
    packed = np.empty((B, FREE), dtype=bf)
    # 20 ctx rows + 1 doc row -> x part
    packed[:, 0 : 20 * D] = we[il[:, 0:20]].reshape(B, -1).astype(bf)
    packed[:, 20 * D : CTX] = de[il[:, 20]].astype(bf)
    # y part: negated target row first, then the 10 noise rows
    packed[:, CTX : CTX + D] = (-oe[ol]).astype(bf)
    packed[:, CTX + D :] = oe[nz].reshape(B, -1).astype(bf)
    return packed


def kernel(**inputs) -> np.ndarray:
    packed = _prep(inputs)
    nc = _get_nc()
    in_maps = [
        {"packed": packed[c * BC : (c + 1) * BC]} for c in range(NCORES)
    ]
    res = bass_utils.run_bass_kernel_spmd(nc, in_maps, core_ids=list(range(NCORES)))
    global _LAST_RESULTS
    _LAST_RESULTS = res
    total = sum(float(r["partial"][0, 0]) for r in res.results)
    return np.float32(total / B)


# revision 7
# speedup vs baseline: 1.0221x; 1.0066x over previous
"""Bass/Trainium2 kernel for nn_KnowledgeD2V (doc2vec NCE loss).

Computation (see reference):
  doc_ids = input_labels[:, -1]; ctx = input_labels[:, :-1]
  x = doc_embed[doc_ids] + word_embed[ctx].sum(1)              # [B, D]
  y = out_embed[[out_labels, noise]]                           # [B, 1+S, D]
  s = einsum('bd,bkd->bk', x, y)                               # [B, 1+S]
  loss = mean_b( softplus(-s[:,0]) + sum_k softplus(s[:,k>0]) )

Strategy: data-parallel over batch across 8 NeuronCores. The host gathers the
32 embedding rows each batch element touches (20 ctx + 1 doc + 1 target + 10
noise) into one contiguous bf16 stream `packed[B, 4096]` — the target row is
sign-flipped so every loss term is softplus(+s) and the device needs a single
Exp/Ln pair per group. Each core streams its 2048 rows tile-by-tile (128
batch rows per SBUF partition-tile, 8 KB/partition sequential HWDGE DMA at
full line rate), then does ALL the arithmetic on-device:
  - x = sum of the 21 ctx+doc rows, via a log-tree of TensorTensor adds
    (contiguous bf16, hits the DVE 2x packed mode; TensorReduce has no fast
    mode so trees beat reductions),
  - prod = y * x (broadcast TT, 2x),
  - dot-fold prod 128->64->32->16->8 (TT 2x) + final TensorReduce,
  - softplus via one Exp + one Ln(1+e) with accum_out -> per-group partials
    (a manual LoadActFuncSet of the shared Exp+Ln table stops the act-table
    pass from thrashing tables every tile),
  - partition-sum via a ones-vector PE matmul.
The host sums the 8 scalars / B. Indirect (gather) DMA is deliberately not
used: multi-offset indirect DMA mis-executes under this runtime.
"""

import numpy as np
import ml_dtypes

import concourse.bacc as bacc
import concourse.mybir as mybir
import concourse.tile as tile
from concourse import bass_utils

B = 16384
S = 10
K = 1 + S         # 11 score columns
D = 128
P = 128
NCORES = 8
BC = B // NCORES  # 2048 rows per core
T = BC // P       # 16 tiles per core
NWIN = 21         # ctx(20) + doc rows summed into x
CTX = NWIN * D    # 2688
FREE = CTX + K * D  # 4096 bf16 elems per batch row
U = 2             # tiles per DVE instruction group
GROUPS = [1, 1] + [U] * ((T - 2) // U)

F32 = mybir.dt.float32
BF16 = mybir.dt.bfloat16


def _build():
    nc = bacc.Bacc(
        "TRN2", target_bir_lowering=False, debug=False, num_devices=NCORES
    )
    packed_d = nc.dram_tensor("packed", [BC, FREE], BF16, kind="ExternalInput").ap()
    res_d = nc.dram_tensor(
        "partial", [P, len(GROUPS)], F32, kind="ExternalOutput"
    ).ap()

    with tile.TileContext(nc) as tc:
        with (
            tc.tile_pool(name="setup", bufs=1) as sp,
            tc.tile_pool(name="work", bufs=3) as wp,
        ):
            acc = sp.tile([P, len(GROUPS)], F32)

            # Preload the single ACT table that holds BOTH Exp and Ln so the
            # act-table-load pass does not reload a table per activation.
            from concourse.hw_specs import get_activation_tables
            _tabs = list(get_activation_tables(nc.m.arch).values())
            _EXP = mybir.ActivationFunctionType.Exp
            _LN = mybir.ActivationFunctionType.Ln
            _set_id = next(i for i, fs in enumerate(_tabs) if _EXP in fs and _LN in fs)
            nc.scalar.add_instruction(mybir.InstLoadActFuncSet(
                name=nc.get_next_instruction_name(),
                act_func_set_id=_set_id, ins=[], outs=[]))

            def tt(out_t, a, b, op=mybir.AluOpType.add):
                nc.vector.tensor_tensor(out=out_t, in0=a, in1=b, op=op)

            t_next = 0
            for gi, GU in enumerate(GROUPS):
                tiles = [t_next + u for u in range(GU)]
                t_next += GU
                g = wp.tile([P, GU, FREE], BF16, tag=f"g{GU}", bufs=4)
                for u, ti in enumerate(tiles):
                    rows = packed_d[ti * P : (ti + 1) * P, :]
                    # split ctx/y so the x-tree can start after the ctx bytes
                    nc.sync.dma_start(out=g[:, u, 0:CTX], in_=rows[:, 0:CTX])
                    nc.sync.dma_start(out=g[:, u, CTX:FREE], in_=rows[:, CTX:FREE])

                with nc.allow_low_precision(reason="bf16 tree-sum of 21 embeds"):
                    r1 = wp.tile([P, GU, 10 * D], BF16, tag=f"r1_{GU}", bufs=2)
                    tt(r1[:], g[:, :, 0 : 10 * D], g[:, :, 10 * D : 20 * D])
                    r2 = wp.tile([P, GU, 5 * D], BF16, tag=f"r2_{GU}", bufs=2)
                    tt(r2[:], r1[:, :, 0 : 5 * D], r1[:, :, 5 * D : 10 * D])
                    r3 = wp.tile([P, GU, 2 * D], BF16, tag=f"r3_{GU}", bufs=2)
                    tt(r3[:], r2[:, :, 0 : 2 * D], r2[:, :, 2 * D : 4 * D])
                    r4 = wp.tile([P, GU, D], BF16, tag=f"r4_{GU}", bufs=2)
                    tt(r4[:], r3[:, :, 0:D], r3[:, :, D : 2 * D])
                    r5 = wp.tile([P, GU, D], BF16, tag=f"r5_{GU}", bufs=2)
                    tt(r5[:], r4[:], r2[:, :, 4 * D : 5 * D])
                    x = wp.tile([P, GU, D], BF16, tag=f"x_{GU}", bufs=2)
                    tt(x[:], r5[:], g[:, :, 20 * D : 21 * D])

                prod = wp.tile([P, GU, K, D], BF16, tag=f"prod_{GU}", bufs=2)
                nc.vector.tensor_tensor(
                    out=prod[:],
                    in0=g[:, :, CTX:FREE].rearrange("p u (k d) -> p u k d", d=D),
                    in1=x[:].rearrange("p u (o d) -> p u o d", o=1)
                         .to_broadcast([P, GU, K, D]),
                    op=mybir.AluOpType.mult,
                )
                with nc.allow_low_precision(reason="bf16 dot folds; loss avgs 180K terms"):
                    f1 = wp.tile([P, GU, K, 64], BF16, tag=f"f1_{GU}", bufs=2)
                    tt(f1[:], prod[:, :, :, 0:64], prod[:, :, :, 64:128])
                    f2 = wp.tile([P, GU, K, 32], BF16, tag=f"f2_{GU}", bufs=2)
                    tt(f2[:], f1[:, :, :, 0:32], f1[:, :, :, 32:64])
                    f3 = wp.tile([P, GU, K, 16], BF16, tag=f"f3_{GU}", bufs=2)
                    tt(f3[:], f2[:, :, :, 0:16], f2[:, :, :, 16:32])
                    f4 = wp.tile([P, GU, K, 8], BF16, tag=f"f4_{GU}", bufs=2)
                    tt(f4[:], f3[:, :, :, 0:8], f3[:, :, :, 8:16])
                v = wp.tile([P, GU * K], F32, tag=f"v_{GU}", bufs=2)
                nc.vector.reduce_sum(
                    out=v[:].rearrange("p (u k) -> p u k", u=GU),
                    in_=f4[:], axis=mybir.AxisListType.X)

                e = wp.tile([P, GU * K], F32, tag=f"e_{GU}", bufs=2)
                nc.scalar.activation(out=e[:], in_=v[:],
                                     func=mybir.ActivationFunctionType.Exp)
                spt = wp.tile([P, GU * K], F32, tag=f"sp_{GU}", bufs=2)
                nc.scalar.activation(out=spt[:], in_=e[:],
                                     func=mybir.ActivationFunctionType.Ln, bias=1.0,
                                     accum_out=acc[:, gi : gi + 1])

            # ship the [P, groups] partial sums; the host adds the 1152
            # floats per core (cheaper than a PE reduction tail on-device)
            nc.sync.dma_start(out=res_d, in_=acc[:])

    nc.compile()
    return nc


_NC = None
_LAST_RESULTS = None  # BassKernelResults of the most recent run (for test harness)


def _get_nc():
    global _NC
    if _NC is None:
        _NC = _build()
    return _NC


def _prep(inputs):
    il = np.asarray(inputs["input_labels"]).astype(np.int64)
    ol = np.asarray(inputs["out_labels"]).astype(np.int64)
    nz = np.asarray(inputs["noise"]).astype(np.int64)
    we = np.asarray(inputs["word_embed"], dtype=np.float32)
    oe = np.asarray(inputs["out_embed"], dtype=np.float32)
    de = np.asarray(inputs["doc_embed"], dtype=np.float32)
    assert int(inputs["num_sampled"]) == S

    bf = ml_dtypes.bfloat16
    packed = np.empty((B, FREE), dtype=bf)
    # 20 ctx rows + 1 doc row -> x part
    packed[:, 0 : 20 * D] = we[il[:, 0:20]].reshape(B, -1).astype(bf)
    packed[:, 20 * D : CTX] = de[il[:, 20]].astype(bf)
    # y part: negated target row first, then the 10 noise rows
    packed[:, CTX : CTX + D] = (-oe[ol]).astype(bf)
    packed[:, CTX + D :] = oe[nz].reshape(B, -1).astype(bf)
    return packed


def kernel(**inputs) -> np.ndarray:
    packed = _prep(inputs)
    nc = _get_nc()
    in_maps = [
        {"packed": packed[c * BC : (c + 1) * BC]} for c in range(NCORES)
    ]
    res = bass_utils.run_bass_kernel_spmd(nc, in_maps, core_ids=list(range(NCORES)))
    global _LAST_RESULTS
    _LAST_RESULTS = res
    total = sum(float(np.asarray(r["partial"], dtype=np.float64).sum())
                for r in res.results)
    return np.float32(total / B)


# revision 8
# speedup vs baseline: 1.0476x; 1.0250x over previous
"""Bass/Trainium2 kernel for nn_KnowledgeD2V (doc2vec NCE loss).

Computation (see reference):
  doc_ids = input_labels[:, -1]; ctx = input_labels[:, :-1]
  x = doc_embed[doc_ids] + word_embed[ctx].sum(1)              # [B, D]
  y = out_embed[[out_labels, noise]]                           # [B, 1+S, D]
  s = einsum('bd,bkd->bk', x, y)                               # [B, 1+S]
  loss = mean_b( softplus(-s[:,0]) + sum_k softplus(s[:,k>0]) )

Strategy: data-parallel over batch across 8 NeuronCores. The host gathers the
32 embedding rows each batch element touches (20 ctx + 1 doc + 1 target + 10
noise) into one contiguous bf16 stream `packed[B, 4096]` — the target row is
sign-flipped so every loss term is softplus(+s) and the device needs a single
Exp/Ln pair per group. Each core streams its 2048 rows tile-by-tile (128
batch rows per SBUF partition-tile, 8 KB/partition sequential HWDGE DMA at
full line rate), then does ALL the arithmetic on-device:
  - x = sum of the 21 ctx+doc rows, via a log-tree of TensorTensor adds
    (contiguous bf16, hits the DVE 2x packed mode; TensorReduce has no fast
    mode so trees beat reductions),
  - prod = y * x (broadcast TT, 2x),
  - dot-fold prod 128->64->32->16->8 (TT 2x) + final TensorReduce,
  - softplus via one Exp + one Ln(1+e) with accum_out -> per-group partials
    (a manual LoadActFuncSet of the shared Exp+Ln table stops the act-table
    pass from thrashing tables every tile),
  - partition-sum via a ones-vector PE matmul.
The host sums the 8 scalars / B. Indirect (gather) DMA is deliberately not
used: multi-offset indirect DMA mis-executes under this runtime.
"""

import numpy as np
import ml_dtypes

import concourse.bacc as bacc
import concourse.mybir as mybir
import concourse.tile as tile
from concourse import bass_utils

B = 16384
S = 10
K = 1 + S         # 11 score columns
D = 128
P = 128
NCORES = 8
BC = B // NCORES  # 2048 rows per core
T = BC // P       # 16 tiles per core
NWIN = 21         # ctx(20) + doc rows summed into x
CTX = NWIN * D    # 2688
FREE = CTX + K * D  # 4096 bf16 elems per batch row
U = 2             # tiles per DVE instruction group
GROUPS = [1, 1] + [U] * ((T - 2) // U)

F32 = mybir.dt.float32
BF16 = mybir.dt.bfloat16
F8 = mybir.dt.float8e4


def _build():
    nc = bacc.Bacc(
        "TRN2", target_bir_lowering=False, debug=False, num_devices=NCORES
    )
    packed_d = nc.dram_tensor("packed", [BC, CTX], BF16, kind="ExternalInput").ap()
    y8_d = nc.dram_tensor("y8", [BC, K * D], F8, kind="ExternalInput").ap()
    res_d = nc.dram_tensor(
        "partial", [P, len(GROUPS)], F32, kind="ExternalOutput"
    ).ap()

    with tile.TileContext(nc) as tc:
        with (
            tc.tile_pool(name="setup", bufs=1) as sp,
            tc.tile_pool(name="work", bufs=3) as wp,
        ):
            acc = sp.tile([P, len(GROUPS)], F32)

            # Preload the single ACT table that holds BOTH Exp and Ln so the
            # act-table-load pass does not reload a table per activation.
            from concourse.hw_specs import get_activation_tables
            _tabs = list(get_activation_tables(nc.m.arch).values())
            _EXP = mybir.ActivationFunctionType.Exp
            _LN = mybir.ActivationFunctionType.Ln
            _set_id = next(i for i, fs in enumerate(_tabs) if _EXP in fs and _LN in fs)
            nc.scalar.add_instruction(mybir.InstLoadActFuncSet(
                name=nc.get_next_instruction_name(),
                act_func_set_id=_set_id, ins=[], outs=[]))

            def tt(out_t, a, b, op=mybir.AluOpType.add):
                nc.vector.tensor_tensor(out=out_t, in0=a, in1=b, op=op)

            t_next = 0
            for gi, GU in enumerate(GROUPS):
                tiles = [t_next + u for u in range(GU)]
                t_next += GU
                g = wp.tile([P, GU, CTX], BF16, tag=f"g{GU}", bufs=4)
                y8 = wp.tile([P, GU, K * D], F8, tag=f"y8_{GU}", bufs=4)
                for u, ti in enumerate(tiles):
                    nc.sync.dma_start(out=g[:, u, :],
                                      in_=packed_d[ti * P : (ti + 1) * P, :])
                    nc.sync.dma_start(out=y8[:, u, :],
                                      in_=y8_d[ti * P : (ti + 1) * P, :])
                # upcast the fp8 y rows on the (otherwise idle) ACT engine;
                # fp8 halves the y stream bytes at no DVE cost
                y = wp.tile([P, GU, K * D], BF16, tag=f"y_{GU}", bufs=2)
                nc.scalar.activation(out=y[:], in_=y8[:],
                                     func=mybir.ActivationFunctionType.Copy)

                with nc.allow_low_precision(reason="bf16 tree-sum of 21 embeds"):
                    r1 = wp.tile([P, GU, 10 * D], BF16, tag=f"r1_{GU}", bufs=2)
                    tt(r1[:], g[:, :, 0 : 10 * D], g[:, :, 10 * D : 20 * D])
                    r2 = wp.tile([P, GU, 5 * D], BF16, tag=f"r2_{GU}", bufs=2)
                    tt(r2[:], r1[:, :, 0 : 5 * D], r1[:, :, 5 * D : 10 * D])
                    r3 = wp.tile([P, GU, 2 * D], BF16, tag=f"r3_{GU}", bufs=2)
                    tt(r3[:], r2[:, :, 0 : 2 * D], r2[:, :, 2 * D : 4 * D])
                    r4 = wp.tile([P, GU, D], BF16, tag=f"r4_{GU}", bufs=2)
                    tt(r4[:], r3[:, :, 0:D], r3[:, :, D : 2 * D])
                    r5 = wp.tile([P, GU, D], BF16, tag=f"r5_{GU}", bufs=2)
                    tt(r5[:], r4[:], r2[:, :, 4 * D : 5 * D])
                    x = wp.tile([P, GU, D], BF16, tag=f"x_{GU}", bufs=2)
                    tt(x[:], r5[:], g[:, :, 20 * D : 21 * D])

                prod = wp.tile([P, GU, K, D], BF16, tag=f"prod_{GU}", bufs=2)
                nc.vector.tensor_tensor(
                    out=prod[:],
                    in0=y[:].rearrange("p u (k d) -> p u k d", d=D),
                    in1=x[:].rearrange("p u (o d) -> p u o d", o=1)
                         .to_broadcast([P, GU, K, D]),
                    op=mybir.AluOpType.mult,
                )
                with nc.allow_low_precision(reason="bf16 dot folds; loss avgs 180K terms"):
                    f1 = wp.tile([P, GU, K, 64], BF16, tag=f"f1_{GU}", bufs=2)
                    tt(f1[:], prod[:, :, :, 0:64], prod[:, :, :, 64:128])
                    f2 = wp.tile([P, GU, K, 32], BF16, tag=f"f2_{GU}", bufs=2)
                    tt(f2[:], f1[:, :, :, 0:32], f1[:, :, :, 32:64])
                    f3 = wp.tile([P, GU, K, 16], BF16, tag=f"f3_{GU}", bufs=2)
                    tt(f3[:], f2[:, :, :, 0:16], f2[:, :, :, 16:32])
                    f4 = wp.tile([P, GU, K, 8], BF16, tag=f"f4_{GU}", bufs=2)
                    tt(f4[:], f3[:, :, :, 0:8], f3[:, :, :, 8:16])
                v = wp.tile([P, GU * K], F32, tag=f"v_{GU}", bufs=2)
                nc.vector.reduce_sum(
                    out=v[:].rearrange("p (u k) -> p u k", u=GU),
                    in_=f4[:], axis=mybir.AxisListType.X)

                e = wp.tile([P, GU * K], F32, tag=f"e_{GU}", bufs=2)
                nc.scalar.activation(out=e[:], in_=v[:],
                                     func=mybir.ActivationFunctionType.Exp)
                spt = wp.tile([P, GU * K], F32, tag=f"sp_{GU}", bufs=2)
                nc.scalar.activation(out=spt[:], in_=e[:],
                                     func=mybir.ActivationFunctionType.Ln, bias=1.0,
                                     accum_out=acc[:, gi : gi + 1])

            # ship the [P, groups] partial sums; the host adds the 1152
            # floats per core (cheaper than a PE reduction tail on-device)
            nc.sync.dma_start(out=res_d, in_=acc[:])

    nc.compile()
    return nc


_NC = None
_LAST_RESULTS = None  # BassKernelResults of the most recent run (for test harness)


def _get_nc():
    global _NC
    if _NC is None:
        _NC = _build()
    return _NC


def _prep(inputs):
    il = np.asarray(inputs["input_labels"]).astype(np.int64)
    ol = np.asarray(inputs["out_labels"]).astype(np.int64)
    nz = np.asarray(inputs["noise"]).astype(np.int64)
    we = np.asarray(inputs["word_embed"], dtype=np.float32)
    oe = np.asarray(inputs["out_embed"], dtype=np.float32)
    de = np.asarray(inputs["doc_embed"], dtype=np.float32)
    assert int(inputs["num_sampled"]) == S

    bf = ml_dtypes.bfloat16
    f8 = ml_dtypes.float8_e4m3
    packed = np.empty((B, CTX), dtype=bf)
    # 20 ctx rows + 1 doc row -> x part (bf16)
    packed[:, 0 : 20 * D] = we[il[:, 0:20]].reshape(B, -1).astype(bf)
    packed[:, 20 * D : CTX] = de[il[:, 20]].astype(bf)
    # y part in fp8 (errors wash out over the 180K averaged loss terms):
    # negated target row first, then the 10 noise rows
    y8 = np.empty((B, K * D), dtype=f8)
    y8[:, 0:D] = (-oe[ol]).astype(f8)
    y8[:, D:] = oe[nz].reshape(B, -1).astype(f8)
    return packed, y8


def kernel(**inputs) -> np.ndarray:
    packed, y8 = _prep(inputs)
    nc = _get_nc()
    in_maps = [
        {"packed": packed[c * BC : (c + 1) * BC],
         "y8": y8[c * BC : (c + 1) * BC]} for c in range(NCORES)
    ]
    res = bass_utils.run_bass_kernel_spmd(nc, in_maps, core_ids=list(range(NCORES)))
    global _LAST_RESULTS
    _LAST_RESULTS = res
    total = sum(float(np.asarray(r["partial"], dtype=np.float64).sum())
                for r in res.results)
    return np.float32(total / B)


# revision 10
# speedup vs baseline: 1.1227x; 1.0717x over previous
"""Bass/Trainium2 kernel for nn_KnowledgeD2V (doc2vec NCE loss).

Computation (see reference):
  doc_ids = input_labels[:, -1]; ctx = input_labels[:, :-1]
  x = doc_embed[doc_ids] + word_embed[ctx].sum(1)              # [B, D]
  y = out_embed[[out_labels, noise]]                           # [B, 1+S, D]
  s = einsum('bd,bkd->bk', x, y)                               # [B, 1+S]
  loss = mean_b( softplus(-s[:,0]) + sum_k softplus(s[:,k>0]) )

Strategy: data-parallel over batch across 8 NeuronCores. The host gathers the
32 embedding rows each batch element touches into two contiguous streams:
`packed[B, 2688]` bf16 (20 ctx + 1 doc rows) and `y8[B, 1408]` fp8-e4m3 (the
target row sign-flipped + 10 noise rows; fp8 errors wash out over the 180K
averaged loss terms). Each core streams its 2048 rows tile-by-tile (128 batch
rows per SBUF partition-tile, sequential HWDGE DMAs at full line rate), then
does ALL the arithmetic on-device:
  - the ACT engine upcasts y fp8->bf16 (activation Copy) — halves the y
    stream bytes at zero DVE cost,
  - x = sum of the 21 ctx+doc rows, via a log-tree of TensorTensor adds
    (contiguous bf16, hits the DVE 2x packed mode; TensorReduce has no fast
    mode so trees beat reductions),
  - prod = y * x (broadcast TT, 2x),
  - dot-fold prod 128->64->32->16->8 (TT 2x) + final TensorReduce,
  - softplus via one Exp + one Ln(1+e) with accum_out -> per-group partials
    (a manual LoadActFuncSet of the shared Exp+Ln table stops the act-table
    pass from thrashing tables every tile).
Each core ships its [128, groups] partial sums; the host sums them / B.
Indirect (gather) DMA is deliberately not used: multi-offset indirect DMA
mis-executes under this runtime.
"""

import numpy as np
import ml_dtypes

import concourse.bacc as bacc
import concourse.mybir as mybir
import concourse.tile as tile
from concourse import bass_utils

B = 16384
S = 10
K = 1 + S         # 11 score columns
D = 128
P = 128
NCORES = 8
BC = B // NCORES  # 2048 rows per core
T = BC // P       # 16 tiles per core
NWIN = 21         # ctx(20) + doc rows summed into x
CTX = NWIN * D    # 2688
FREE = CTX + K * D  # 4096 bf16 elems per batch row
U = 2             # tiles per DVE instruction group
GROUPS = [1, 1] + [U] * ((T - 2) // U)

F32 = mybir.dt.float32
BF16 = mybir.dt.bfloat16
F8 = mybir.dt.float8e4


def _build():
    nc = bacc.Bacc(
        "TRN2", target_bir_lowering=False, debug=False, num_devices=NCORES
    )
    packed_d = nc.dram_tensor("packed", [BC, CTX], BF16, kind="ExternalInput").ap()
    y8_d = nc.dram_tensor("y8", [BC, K * D], F8, kind="ExternalInput").ap()
    res_d = nc.dram_tensor(
        "partial", [P, len(GROUPS)], F32, kind="ExternalOutput"
    ).ap()

    with tile.TileContext(nc) as tc:
        with (
            tc.tile_pool(name="setup", bufs=1) as sp,
            tc.tile_pool(name="work", bufs=3) as wp,
        ):
            acc = sp.tile([P, len(GROUPS)], F32)

            # Preload the single ACT table that holds BOTH Exp and Ln so the
            # act-table-load pass does not reload a table per activation.
            from concourse.hw_specs import get_activation_tables
            _tabs = list(get_activation_tables(nc.m.arch).values())
            _EXP = mybir.ActivationFunctionType.Exp
            _LN = mybir.ActivationFunctionType.Ln
            _set_id = next(i for i, fs in enumerate(_tabs) if _EXP in fs and _LN in fs)
            nc.scalar.add_instruction(mybir.InstLoadActFuncSet(
                name=nc.get_next_instruction_name(),
                act_func_set_id=_set_id, ins=[], outs=[]))

            def tt(out_t, a, b, op=mybir.AluOpType.add):
                nc.vector.tensor_tensor(out=out_t, in0=a, in1=b, op=op)

            t_next = 0
            for gi, GU in enumerate(GROUPS):
                tiles = [t_next + u for u in range(GU)]
                t_next += GU
                g = wp.tile([P, GU, CTX], BF16, tag=f"g{GU}", bufs=4)
                y8 = wp.tile([P, GU, K * D], F8, tag=f"y8_{GU}", bufs=4)
                for u, ti in enumerate(tiles):
                    # y8 first: its ACT upcast then overlaps the ctx stream
                    nc.sync.dma_start(out=y8[:, u, :],
                                      in_=y8_d[ti * P : (ti + 1) * P, :])
                    nc.sync.dma_start(out=g[:, u, :],
                                      in_=packed_d[ti * P : (ti + 1) * P, :])
                # upcast the fp8 y rows on the (otherwise idle) ACT engine;
                # fp8 halves the y stream bytes at no DVE cost
                y = wp.tile([P, GU, K * D], BF16, tag=f"y_{GU}", bufs=2)
                nc.scalar.activation(out=y[:], in_=y8[:],
                                     func=mybir.ActivationFunctionType.Copy)

                with nc.allow_low_precision(reason="bf16 tree-sum of 21 embeds"):
                    r1 = wp.tile([P, GU, 10 * D], BF16, tag=f"r1_{GU}", bufs=2)
                    tt(r1[:], g[:, :, 0 : 10 * D], g[:, :, 10 * D : 20 * D])
                    r2 = wp.tile([P, GU, 5 * D], BF16, tag=f"r2_{GU}", bufs=2)
                    tt(r2[:], r1[:, :, 0 : 5 * D], r1[:, :, 5 * D : 10 * D])
                    r3 = wp.tile([P, GU, 2 * D], BF16, tag=f"r3_{GU}", bufs=2)
                    tt(r3[:], r2[:, :, 0 : 2 * D], r2[:, :, 2 * D : 4 * D])
                    r4 = wp.tile([P, GU, D], BF16, tag=f"r4_{GU}", bufs=2)
                    tt(r4[:], r3[:, :, 0:D], r3[:, :, D : 2 * D])
                    r5 = wp.tile([P, GU, D], BF16, tag=f"r5_{GU}", bufs=2)
                    tt(r5[:], r4[:], r2[:, :, 4 * D : 5 * D])
                    x = wp.tile([P, GU, D], BF16, tag=f"x_{GU}", bufs=2)
                    tt(x[:], r5[:], g[:, :, 20 * D : 21 * D])

                prod = wp.tile([P, GU, K, D], BF16, tag=f"prod_{GU}", bufs=2)
                nc.vector.tensor_tensor(
                    out=prod[:],
                    in0=y[:].rearrange("p u (k d) -> p u k d", d=D),
                    in1=x[:].rearrange("p u (o d) -> p u o d", o=1)
                         .to_broadcast([P, GU, K, D]),
                    op=mybir.AluOpType.mult,
                )
                with nc.allow_low_precision(reason="bf16 dot folds; loss avgs 180K terms"):
                    # f1 on DVE; the remaining folds run on the otherwise-idle
                    # GPSIMD engine. The pipeline is strictly forward
                    # (DVE -> Pool -> ACT) so no engine stalls on a later
                    # stage. The LAST group folds on DVE instead, skipping the
                    # Pool hop that would otherwise lengthen the drain tail.
                    last = gi == len(GROUPS) - 1
                    eng = nc.vector if last else nc.gpsimd
                    def pt(out_t, a, b):
                        eng.tensor_tensor(out=out_t, in0=a, in1=b,
                                          op=mybir.AluOpType.add)
                    f1 = wp.tile([P, GU, K, 64], BF16, tag=f"f1_{GU}", bufs=2)
                    tt(f1[:], prod[:, :, :, 0:64], prod[:, :, :, 64:128])
                    f2 = wp.tile([P, GU, K, 32], BF16, tag=f"f2_{GU}", bufs=2)
                    pt(f2[:], f1[:, :, :, 0:32], f1[:, :, :, 32:64])
                    f3 = wp.tile([P, GU, K, 16], BF16, tag=f"f3_{GU}", bufs=2)
                    pt(f3[:], f2[:, :, :, 0:16], f2[:, :, :, 16:32])
                    f4 = wp.tile([P, GU, K, 8], BF16, tag=f"f4_{GU}", bufs=2)
                    pt(f4[:], f3[:, :, :, 0:8], f3[:, :, :, 8:16])
                    f5 = wp.tile([P, GU, K, 4], BF16, tag=f"f5_{GU}", bufs=2)
                    pt(f5[:], f4[:, :, :, 0:4], f4[:, :, :, 4:8])
                    f6 = wp.tile([P, GU, K, 2], BF16, tag=f"f6_{GU}", bufs=2)
                    pt(f6[:], f5[:, :, :, 0:2], f5[:, :, :, 2:4])
                v = wp.tile([P, GU * K], F32, tag=f"v_{GU}", bufs=2)
                eng.tensor_tensor(
                    out=v[:].rearrange("p (u k) -> p u k", u=GU),
                    in0=f6[:, :, :, 0].rearrange("p u k -> p u k"),
                    in1=f6[:, :, :, 1].rearrange("p u k -> p u k"),
                    op=mybir.AluOpType.add)

                e = wp.tile([P, GU * K], F32, tag=f"e_{GU}", bufs=2)
                nc.scalar.activation(out=e[:], in_=v[:],
                                     func=mybir.ActivationFunctionType.Exp)
                spt = wp.tile([P, GU * K], F32, tag=f"sp_{GU}", bufs=2)
                nc.scalar.activation(out=spt[:], in_=e[:],
                                     func=mybir.ActivationFunctionType.Ln, bias=1.0,
                                     accum_out=acc[:, gi : gi + 1])

            # ship the [P, groups] partial sums; the host adds the 1152
            # floats per core (cheaper than a PE reduction tail on-device)
            nc.sync.dma_start(out=res_d, in_=acc[:])

    nc.compile()
    return nc


_NC = None
_LAST_RESULTS = None  # BassKernelResults of the most recent run (for test harness)


def _get_nc():
    global _NC
    if _NC is None:
        _NC = _build()
    return _NC


def _prep(inputs):
    il = np.asarray(inputs["input_labels"]).astype(np.int64)
    ol = np.asarray(inputs["out_labels"]).astype(np.int64)
    nz = np.asarray(inputs["noise"]).astype(np.int64)
    we = np.asarray(inputs["word_embed"], dtype=np.float32)
    oe = np.asarray(inputs["out_embed"], dtype=np.float32)
    de = np.asarray(inputs["doc_embed"], dtype=np.float32)
    assert int(inputs["num_sampled"]) == S

    bf = ml_dtypes.bfloat16
    f8 = ml_dtypes.float8_e4m3
    packed = np.empty((B, CTX), dtype=bf)
    # 20 ctx rows + 1 doc row -> x part (bf16)
    packed[:, 0 : 20 * D] = we[il[:, 0:20]].reshape(B, -1).astype(bf)
    packed[:, 20 * D : CTX] = de[il[:, 20]].astype(bf)
    # y part in fp8 (errors wash out over the 180K averaged loss terms):
    # negated target row first, then the 10 noise rows
    y8 = np.empty((B, K * D), dtype=f8)
    y8[:, 0:D] = (-oe[ol]).astype(f8)
    y8[:, D:] = oe[nz].reshape(B, -1).astype(f8)
    return packed, y8


def kernel(**inputs) -> np.ndarray:
    packed, y8 = _prep(inputs)
    nc = _get_nc()
    in_maps = [
        {"packed": packed[c * BC : (c + 1) * BC],
         "y8": y8[c * BC : (c + 1) * BC]} for c in range(NCORES)
    ]
    res = bass_utils.run_bass_kernel_spmd(nc, in_maps, core_ids=list(range(NCORES)))
    global _LAST_RESULTS
    _LAST_RESULTS = res
    total = sum(float(np.asarray(r["partial"], dtype=np.float64).sum())
                for r in res.results)
    return np.float32(total / B)


# revision 12
# speedup vs baseline: 1.1255x; 1.0025x over previous
"""Bass/Trainium2 kernel for nn_KnowledgeD2V (doc2vec NCE loss).

Computation (see reference):
  doc_ids = input_labels[:, -1]; ctx = input_labels[:, :-1]
  x = doc_embed[doc_ids] + word_embed[ctx].sum(1)              # [B, D]
  y = out_embed[[out_labels, noise]]                           # [B, 1+S, D]
  s = einsum('bd,bkd->bk', x, y)                               # [B, 1+S]
  loss = mean_b( softplus(-s[:,0]) + sum_k softplus(s[:,k>0]) )

Strategy: data-parallel over batch across 8 NeuronCores. The host gathers the
32 embedding rows each batch element touches into two contiguous streams:
`packed[B, 2688]` bf16 (20 ctx + 1 doc rows) and `y8[B, 1408]` fp8-e4m3 (the
target row sign-flipped + 10 noise rows; fp8 errors wash out over the 180K
averaged loss terms). Each core streams its 2048 rows tile-by-tile (128 batch
rows per SBUF partition-tile, sequential HWDGE DMAs at full line rate), then
does ALL the arithmetic on-device:
  - the ACT engine upcasts y fp8->bf16 (activation Copy) — halves the y
    stream bytes at zero DVE cost,
  - x = sum of the 21 ctx+doc rows, via a log-tree of TensorTensor adds
    (contiguous bf16, hits the DVE 2x packed mode; TensorReduce has no fast
    mode so trees beat reductions),
  - prod = y * x (broadcast TT, 2x) and the first dot-fold on DVE; the
    remaining folds (64->...->1) run on the otherwise-idle GPSIMD engine as a
    strictly forward DVE -> Pool -> ACT pipeline (no engine ever waits on a
    later stage; the last group folds on DVE to keep the drain tail short),
  - softplus via one Exp + one Ln(1+e) with accum_out -> per-group partials
    (a manual LoadActFuncSet of the shared Exp+Ln table stops the act-table
    pass from thrashing tables every tile).
Each core ships its [128, groups] partial sums; the host sums them / B.
Indirect (gather) DMA is deliberately not used: multi-offset indirect DMA
mis-executes under this runtime.
"""

import numpy as np
import ml_dtypes

import concourse.bacc as bacc
import concourse.mybir as mybir
import concourse.tile as tile
from concourse import bass_utils

B = 16384
S = 10
K = 1 + S         # 11 score columns
D = 128
P = 128
NCORES = 8
BC = B // NCORES  # 2048 rows per core
T = BC // P       # 16 tiles per core
NWIN = 21         # ctx(20) + doc rows summed into x
CTX = NWIN * D    # 2688
FREE = CTX + K * D  # 4096 bf16 elems per batch row
U = 2             # tiles per DVE instruction group
GROUPS = [1, 1] + [U] * ((T - 2) // U)

F32 = mybir.dt.float32
BF16 = mybir.dt.bfloat16
F8 = mybir.dt.float8e4


def _build():
    nc = bacc.Bacc(
        "TRN2", target_bir_lowering=False, debug=False, num_devices=NCORES
    )
    packed_d = nc.dram_tensor("packed", [BC, CTX], BF16, kind="ExternalInput").ap()
    y8_d = nc.dram_tensor("y8", [BC, K * D], F8, kind="ExternalInput").ap()
    res_d = nc.dram_tensor(
        "partial", [P, len(GROUPS)], F32, kind="ExternalOutput"
    ).ap()

    with tile.TileContext(nc) as tc:
        with (
            tc.tile_pool(name="setup", bufs=1) as sp,
            tc.tile_pool(name="work", bufs=3) as wp,
        ):
            acc = sp.tile([P, len(GROUPS)], F32)

            # Preload the single ACT table that holds BOTH Exp and Ln so the
            # act-table-load pass does not reload a table per activation.
            from concourse.hw_specs import get_activation_tables
            _tabs = list(get_activation_tables(nc.m.arch).values())
            _EXP = mybir.ActivationFunctionType.Exp
            _LN = mybir.ActivationFunctionType.Ln
            _set_id = next(i for i, fs in enumerate(_tabs) if _EXP in fs and _LN in fs)
            nc.scalar.add_instruction(mybir.InstLoadActFuncSet(
                name=nc.get_next_instruction_name(),
                act_func_set_id=_set_id, ins=[], outs=[]))

            def tt(out_t, a, b, op=mybir.AluOpType.add):
                nc.vector.tensor_tensor(out=out_t, in0=a, in1=b, op=op)

            t_next = 0
            for gi, GU in enumerate(GROUPS):
                tiles = [t_next + u for u in range(GU)]
                t_next += GU
                g = wp.tile([P, GU, CTX], BF16, tag=f"g{GU}", bufs=4)
                y8 = wp.tile([P, GU, K * D], F8, tag=f"y8_{GU}", bufs=4)
                for u, ti in enumerate(tiles):
                    # y8 first so its ACT upcast overlaps the ctx stream —
                    # except group 0, where ctx-first shortens the ramp (ACT
                    # has full slack before any softplus work exists)
                    if gi == 0:
                        nc.sync.dma_start(out=g[:, u, :],
                                          in_=packed_d[ti * P : (ti + 1) * P, :])
                        nc.sync.dma_start(out=y8[:, u, :],
                                          in_=y8_d[ti * P : (ti + 1) * P, :])
                    else:
                        nc.sync.dma_start(out=y8[:, u, :],
                                          in_=y8_d[ti * P : (ti + 1) * P, :])
                        nc.sync.dma_start(out=g[:, u, :],
                                          in_=packed_d[ti * P : (ti + 1) * P, :])
                # upcast the fp8 y rows on the (otherwise idle) ACT engine;
                # fp8 halves the y stream bytes at no DVE cost
                y = wp.tile([P, GU, K * D], BF16, tag=f"y_{GU}", bufs=2)
                nc.scalar.activation(out=y[:], in_=y8[:],
                                     func=mybir.ActivationFunctionType.Copy)

                with nc.allow_low_precision(reason="bf16 tree-sum of 21 embeds"):
                    r1 = wp.tile([P, GU, 10 * D], BF16, tag=f"r1_{GU}", bufs=2)
                    tt(r1[:], g[:, :, 0 : 10 * D], g[:, :, 10 * D : 20 * D])
                    r2 = wp.tile([P, GU, 5 * D], BF16, tag=f"r2_{GU}", bufs=2)
                    tt(r2[:], r1[:, :, 0 : 5 * D], r1[:, :, 5 * D : 10 * D])
                    r3 = wp.tile([P, GU, 2 * D], BF16, tag=f"r3_{GU}", bufs=2)
                    tt(r3[:], r2[:, :, 0 : 2 * D], r2[:, :, 2 * D : 4 * D])
                    r4 = wp.tile([P, GU, D], BF16, tag=f"r4_{GU}", bufs=2)
                    tt(r4[:], r3[:, :, 0:D], r3[:, :, D : 2 * D])
                    r5 = wp.tile([P, GU, D], BF16, tag=f"r5_{GU}", bufs=2)
                    tt(r5[:], r4[:], r2[:, :, 4 * D : 5 * D])
                    x = wp.tile([P, GU, D], BF16, tag=f"x_{GU}", bufs=2)
                    tt(x[:], r5[:], g[:, :, 20 * D : 21 * D])

                prod = wp.tile([P, GU, K, D], BF16, tag=f"prod_{GU}", bufs=2)
                nc.vector.tensor_tensor(
                    out=prod[:],
                    in0=y[:].rearrange("p u (k d) -> p u k d", d=D),
                    in1=x[:].rearrange("p u (o d) -> p u o d", o=1)
                         .to_broadcast([P, GU, K, D]),
                    op=mybir.AluOpType.mult,
                )
                with nc.allow_low_precision(reason="bf16 dot folds; loss avgs 180K terms"):
                    # f1 on DVE; the remaining folds run on the otherwise-idle
                    # GPSIMD engine. The pipeline is strictly forward
                    # (DVE -> Pool -> ACT) so no engine stalls on a later
                    # stage. The LAST group folds on DVE instead, skipping the
                    # Pool hop that would otherwise lengthen the drain tail.
                    last = gi == len(GROUPS) - 1
                    eng = nc.vector if last else nc.gpsimd
                    def pt(out_t, a, b):
                        eng.tensor_tensor(out=out_t, in0=a, in1=b,
                                          op=mybir.AluOpType.add)
                    f1 = wp.tile([P, GU, K, 64], BF16, tag=f"f1_{GU}", bufs=2)
                    tt(f1[:], prod[:, :, :, 0:64], prod[:, :, :, 64:128])
                    f2 = wp.tile([P, GU, K, 32], BF16, tag=f"f2_{GU}", bufs=2)
                    pt(f2[:], f1[:, :, :, 0:32], f1[:, :, :, 32:64])
                    f3 = wp.tile([P, GU, K, 16], BF16, tag=f"f3_{GU}", bufs=2)
                    pt(f3[:], f2[:, :, :, 0:16], f2[:, :, :, 16:32])
                    f4 = wp.tile([P, GU, K, 8], BF16, tag=f"f4_{GU}", bufs=2)
                    pt(f4[:], f3[:, :, :, 0:8], f3[:, :, :, 8:16])
                    f5 = wp.tile([P, GU, K, 4], BF16, tag=f"f5_{GU}", bufs=2)
                    pt(f5[:], f4[:, :, :, 0:4], f4[:, :, :, 4:8])
                    f6 = wp.tile([P, GU, K, 2], BF16, tag=f"f6_{GU}", bufs=2)
                    pt(f6[:], f5[:, :, :, 0:2], f5[:, :, :, 2:4])
                v = wp.tile([P, GU * K], F32, tag=f"v_{GU}", bufs=2)
                eng.tensor_tensor(
                    out=v[:].rearrange("p (u k) -> p u k", u=GU),
                    in0=f6[:, :, :, 0].rearrange("p u k -> p u k"),
                    in1=f6[:, :, :, 1].rearrange("p u k -> p u k"),
                    op=mybir.AluOpType.add)

                e = wp.tile([P, GU * K], F32, tag=f"e_{GU}", bufs=2)
                nc.scalar.activation(out=e[:], in_=v[:],
                                     func=mybir.ActivationFunctionType.Exp)
                spt = wp.tile([P, GU * K], F32, tag=f"sp_{GU}", bufs=2)
                nc.scalar.activation(out=spt[:], in_=e[:],
                                     func=mybir.ActivationFunctionType.Ln, bias=1.0,
                                     accum_out=acc[:, gi : gi + 1])

            # ship the [P, groups] partial sums; the host adds the 1152
            # floats per core (cheaper than a PE reduction tail on-device)
            nc.sync.dma_start(out=res_d, in_=acc[:])

    nc.compile()
    return nc


_NC = None
_LAST_RESULTS = None  # BassKernelResults of the most recent run (for test harness)


def _get_nc():
    global _NC
    if _NC is None:
        _NC = _build()
    return _NC


def _prep(inputs):
    il = np.asarray(inputs["input_labels"]).astype(np.int64)
    ol = np.asarray(inputs["out_labels"]).astype(np.int64)
    nz = np.asarray(inputs["noise"]).astype(np.int64)
    we = np.asarray(inputs["word_embed"], dtype=np.float32)
    oe = np.asarray(inputs["out_embed"], dtype=np.float32)
    de = np.asarray(inputs["doc_embed"], dtype=np.float32)
    assert int(inputs["num_sampled"]) == S

    bf = ml_dtypes.bfloat16
    f8 = ml_dtypes.float8_e4m3
    packed = np.empty((B, CTX), dtype=bf)
    # 20 ctx rows + 1 doc row -> x part (bf16)
    packed[:, 0 : 20 * D] = we[il[:, 0:20]].reshape(B, -1).astype(bf)
    packed[:, 20 * D : CTX] = de[il[:, 20]].astype(bf)
    # y part in fp8 (errors wash out over the 180K averaged loss terms):
    # negated target row first, then the 10 noise rows
    y8 = np.empty((B, K * D), dtype=f8)
    y8[:, 0:D] = (-oe[ol]).astype(f8)
    y8[:, D:] = oe[nz].reshape(B, -1).astype(f8)
    return packed, y8


def kernel(**inputs) -> np.ndarray:
    packed, y8 = _prep(inputs)
    nc = _get_nc()
    in_maps = [
        {"packed": packed[c * BC : (c + 1) * BC],
         "y8": y8[c * BC : (c + 1) * BC]} for c in range(NCORES)
    ]
    res = bass_utils.run_bass_kernel_spmd(nc, in_maps, core_ids=list(range(NCORES)))
    global _LAST_RESULTS
    _LAST_RESULTS = res
    total = sum(float(np.asarray(r["partial"], dtype=np.float64).sum())
                for r in res.results)
    return np.float32(total / B)
